# revision 18
# baseline (speedup 1.0000x reference)
"""Trainium2 Bass kernel for a cross-attention transformer layer.

Reference computation (per batch b):
    Q = query @ Wq.T ; K = key @ Wk.T ; V = value @ Wv.T
    scores = QK^T/sqrt(d_k) per head, masked, softmax
    out = LayerNorm(softmax(scores) V @ Wo.T + query)

Sharding: 8 cores = 4 batches x 2 query-halves; no collectives.

All matmuls run in fp8 (e4m3 operands) DoubleRow perf mode: each matmul
contracts 2x128 (two "planes" in the free dim of both operands) at 0.5
cycles per output column. Layouts (per core, device side):
  qT/kT/vT inputs  [128, s2, cp2, seq]   c_in = 256*s + 128*cp + p
  wq/wk/wv (DR)    [128, s2, cp2, 512]   plain chunked W.T
  QT/KT            [128, g4, seq] bf16; partition 64*(h%2)+dk, g = h//2.
                   Scores are plain bf16 matmuls (K=64, base 0/64; DR is
                   impossible for a 64-deep contraction since matmul
                   operand base partitions are restricted to {0,32,64})
  V (+ones col)    [128, ktp8, vpl2, h*65+e]  ctx DR contracts 2 k-tiles
                   per step; the ones column makes psum row 64 the softmax
                   denominator for free
  PT = exp(s/8)    e5m2 [128, 2, 1024] per k-tile pair; written either by
                   ACT (table exp, fp8 out) or DVE (Schraudolph: the int8
                   of round(s*0.72135 + 59.75) IS the e5m2 bit pattern;
                   mask rides in the per-partition scale/offset tables)
  ctx              [64, pair4, h2, q] e4m3, scaled x64/denominator at
                   eviction (Pool broadcasts 1/den across partitions)
  O-proj           DR over pairs (K=64 x 2 head planes), /64 folded into
                   the residual add; LayerNorm: DVE stats + Pool apply.
"""

import sys

if "/opt/trn_rl_repo" not in sys.path:
    sys.path.insert(0, "/opt/trn_rl_repo")

import numpy as np
import ml_dtypes

import concourse.bacc as bacc
import concourse.mybir as mybir
import concourse.tile as tile
from concourse import bass_utils

F32 = mybir.dt.float32
BF16 = mybir.dt.bfloat16
FP8E4 = mybir.dt.float8e4
FP8E5 = mybir.dt.float8e5
I8 = mybir.dt.int8
AF = mybir.ActivationFunctionType
ALU = mybir.AluOpType
PM = mybir.MatmulPerfMode

E4 = ml_dtypes.float8_e4m3
E5 = ml_dtypes.float8_e5m2

D_MODEL = 512
N_HEADS = 8
D_K = 64
SQ = 1024          # query rows per core
SK = 2048          # key rows per core
N_CORES = 8
P = 128

# Schraudolph constants for e5m2 bits: round(x*(4/ln2)*0.125 + 60 - C)
SCH_A = 0.125 * 4.0 / np.log(2.0)
SCH_B = 59.75
ACT_MASK_BIAS = -30.0          # exp(-30) underflows e5m2 -> exact 0
CTX_SCALE = 64.0

# exp engine assignment per kt (per head): True -> ACT, False -> DVE.
# Interleaved within kt-pairs so both engines work concurrently.
EXP_ON_ACT = [True, False] * 8

_NC_CACHE: dict = {}


def _build(qkv_bias: bool, ln_affine: bool):
    nc = bacc.Bacc("TRN2", target_bir_lowering=False, debug=False,
                   enable_asserts=False, num_devices=N_CORES)

    d = lambda name, shape, dt: nc.dram_tensor(name, shape, dt, kind="ExternalInput").ap()
    qT = d("qT", [P, 2, 2, SQ], FP8E4)
    kT = d("kT", [P, 2, 2, SK], FP8E4)
    vT = d("vT", [P, 2, 2, SK], FP8E4)
    wq = d("wq", [P, 2, 2, D_MODEL], FP8E4)
    wk = d("wk", [P, 2, 2, D_MODEL], FP8E4)
    wv = d("wv", [P, 2, 2, D_MODEL], FP8E4)
    wo = d("wo", [D_K, 4, 2, D_MODEL], FP8E4)
    at = d("at", [P, 16], F32)       # schraudolph scale per (key, kt)
    bt = d("bt", [P, 16], F32)       # schraudolph offset per (key, kt)
    mt = d("mt", [P, 16], F32)       # ACT exp bias (mask)
    qres = d("qres", [SQ, D_MODEL], F32)
    if qkv_bias:
        bqp = d("bqp", [P, 4], F32)      # bias chunked to QT partition order
        bkp = d("bkp", [P, 4], F32)
        bvb = d("bvb", [P, D_MODEL], F32)
    if ln_affine:
        gam = d("gam", [P, D_MODEL], F32)
        bet = d("bet", [P, D_MODEL], F32)
    out = nc.dram_tensor("out", [SQ, D_MODEL], F32, kind="ExternalOutput").ap()

    NQC = SQ // P      # 8 q chunks for O-proj/LN
    KT_TILES = SK // P  # 16
    NKTP = KT_TILES // 2

    with tile.TileContext(nc) as tc:
        with (
            tc.tile_pool(name="consts", bufs=1) as consts,
            tc.tile_pool(name="inbuf", bufs=1) as inbuf,
            tc.tile_pool(name="acts", bufs=1) as acts,
            tc.tile_pool(name="ptp", bufs=3) as ptp,
            tc.tile_pool(name="nrm", bufs=2) as nrm,
            tc.tile_pool(name="lnp", bufs=2) as lnp,
            tc.tile_pool(name="small", bufs=4) as small,
            # PSUM: "sc" 2x[128,1024]f32 = 4 banks, "ctx" 2x = 4 banks
            tc.tile_pool(name="ps_sc", bufs=2, space="PSUM") as ps_sc,
            tc.tile_pool(name="ps_ctx", bufs=2, space="PSUM") as ps_ctx,
        ):
            # ---- loads ----------------------------------------------------
            wq_sb = consts.tile([P, 2, 2, D_MODEL], FP8E4, tag="wq")
            nc.sync.dma_start(wq_sb[:], wq)
            qT_sb = inbuf.tile([P, 2, 2, SQ], FP8E4, tag="qT")
            nc.sync.dma_start(qT_sb[:], qT)
            at_sb = consts.tile([P, 16], F32, tag="at")
            nc.sync.dma_start(at_sb[:], at)
            bt_sb = consts.tile([P, 16], F32, tag="bt")
            nc.sync.dma_start(bt_sb[:], bt)
            mt_sb = consts.tile([P, 16], F32, tag="mt")
            nc.sync.dma_start(mt_sb[:], mt)
            wk_sb = consts.tile([P, 2, 2, D_MODEL], FP8E4, tag="wk")
            nc.sync.dma_start(wk_sb[:], wk)
            kT_sb = inbuf.tile([P, 2, 2, SK], FP8E4, tag="kT")
            nc.sync.dma_start(kT_sb[:], kT)
            wv_sb = consts.tile([P, 2, 2, D_MODEL], FP8E4, tag="wv")
            nc.sync.dma_start(wv_sb[:], wv)
            vT_sb = inbuf.tile([P, 2, 2, SK], FP8E4, tag="vT")
            nc.sync.dma_start(vT_sb[:], vT)
            wo_sb = consts.tile([D_K, 4, 2, D_MODEL], FP8E4, tag="wo")
            nc.sync.dma_start(wo_sb[:], wo)
            qres_sb = inbuf.tile([P, NQC, D_MODEL], F32, tag="qres")
            nc.sync.dma_start(qres_sb[:],
                              qres.rearrange("(qc p) o -> p qc o", p=P))
            if qkv_bias:
                bqp_sb = consts.tile([P, 4], F32, tag="bqp")
                nc.sync.dma_start(bqp_sb[:], bqp)
                bkp_sb = consts.tile([P, 4], F32, tag="bkp")
                nc.sync.dma_start(bkp_sb[:], bkp)
                bvb_sb = consts.tile([P, D_MODEL], F32, tag="bvb")
                nc.sync.dma_start(bvb_sb[:], bvb)
            if ln_affine:
                gam_sb = consts.tile([P, D_MODEL], F32, tag="gam")
                nc.sync.dma_start(gam_sb[:], gam)
                bet_sb = consts.tile([P, D_MODEL], F32, tag="bet")
                nc.sync.dma_start(bet_sb[:], bet)
            eps_sb = consts.tile([P, 1], F32, tag="eps")
            nc.gpsimd.memset(eps_sb[:], 1e-5)

            # V padded to 96 cols/head (DR weights need M % 32 == 0):
            # e<64 = V, e=64 ones (softmax denominator row), e>64 zeros
            VW = 96
            v_sb = acts.tile([P, NKTP, 2, N_HEADS * VW], FP8E4, tag="v")
            v_by_head = v_sb[:].rearrange("p a b (h e) -> p a b h e", e=VW)
            nc.gpsimd.memset(v_by_head[:, :, :, :, 64:65], 1.0)
            nc.gpsimd.memset(v_by_head[:, :, :, :, 65:VW], 0.0)

            evict_tick = [0]

            def evict(dst, src, bias_ap=None):
                """PSUM f32 -> SBUF convert, alternating ACT/DVE."""
                use_act = evict_tick[0] % 2 == 0
                evict_tick[0] += 1
                if bias_ap is None:
                    if use_act:
                        nc.scalar.copy(dst, src)
                    else:
                        nc.vector.tensor_copy(dst, src)
                else:
                    if use_act:
                        nc.scalar.activation(dst, src, AF.Identity, bias=bias_ap)
                    else:
                        nc.vector.tensor_scalar_add(dst, src, bias_ap)

            # ---- projections (fp8 DR, M=128) ------------------------------
            QT_s = acts.tile([P, 4, SQ], BF16, tag="QT")
            for g in range(4):
                for q0 in range(0, SQ, 512):
                    psq = ps_sc.tile([P, 512], F32, tag="sc", name="psq")
                    for s in range(2):
                        nc.tensor.matmul(psq[:], wq_sb[:, s, :, g * P:(g + 1) * P],
                                         qT_sb[:, s, :, q0:q0 + 512],
                                         start=(s == 0), stop=(s == 1),
                                         perf_mode=PM.DoubleRow)
                    evict(QT_s[:, g, q0:q0 + 512], psq[:],
                          bqp_sb[:, g:g + 1] if qkv_bias else None)

            KT_s = acts.tile([P, 4, SK], BF16, tag="KT")
            for g in range(4):
                for k0 in range(0, SK, 512):
                    psk = ps_sc.tile([P, 512], F32, tag="sc", name="psk")
                    for s in range(2):
                        nc.tensor.matmul(psk[:], wk_sb[:, s, :, g * P:(g + 1) * P],
                                         kT_sb[:, s, :, k0:k0 + 512],
                                         start=(s == 0), stop=(s == 1),
                                         perf_mode=PM.DoubleRow)
                    evict(KT_s[:, g, k0:k0 + 512], psk[:],
                          bkp_sb[:, g:g + 1] if qkv_bias else None)

            for kt in range(KT_TILES):
                psv = ps_sc.tile([P, 512], F32, tag="sc", name="psv")
                for s in range(2):
                    nc.tensor.matmul(psv[:], vT_sb[:, s, :, kt * P:(kt + 1) * P],
                                     wv_sb[:, s, :, :],
                                     start=(s == 0), stop=(s == 1),
                                     perf_mode=PM.DoubleRow)
                dst = v_sb[:, kt // 2, kt % 2, :].rearrange(
                    "p (h e) -> p h e", e=VW)[:, :, 0:64]
                src = psv[:].rearrange("p (h e) -> p h e", e=64)
                if qkv_bias:
                    nc.vector.scalar_tensor_tensor(
                        dst, src, 1.0,
                        bvb_sb[:].rearrange("p (h e) -> p h e", e=64),
                        ALU.mult, ALU.add)
                else:
                    evict(dst, src)

            # ---- attention ------------------------------------------------
            recips = nrm  # [1,1024] f32 + [64,1024] f32 tiles
            for h in range(N_HEADS):
                g, hb = h // 2, h % 2
                b0 = D_K * hb
                ctx_ps = ps_ctx.tile([P, SQ], F32, tag="ctx", name=f"ctx_h{h}")
                pt = None
                for kt in range(KT_TILES):
                    if kt % 2 == 0:
                        pt = ptp.tile([P, 2, SQ], FP8E5, tag="pt",
                                      name=f"pt_h{h}_{kt // 2}")
                    sc = ps_sc.tile([P, SQ], F32, tag="sc", name=f"sc_h{h}_{kt}")
                    for q0 in range(0, SQ, 512):
                        nc.tensor.matmul(
                            sc[:, q0:q0 + 512],
                            KT_s[b0:b0 + D_K, g, kt * P:(kt + 1) * P],
                            QT_s[b0:b0 + D_K, g, q0:q0 + 512],
                            start=True, stop=True)
                    dst = pt[:, kt % 2, :]
                    if EXP_ON_ACT[kt]:
                        nc.scalar.activation(dst, sc[:], AF.Exp,
                                             bias=mt_sb[:, kt:kt + 1],
                                             scale=0.125)
                    else:
                        nc.vector.tensor_scalar(
                            dst.bitcast(I8), sc[:],
                            at_sb[:, kt:kt + 1], bt_sb[:, kt:kt + 1],
                            ALU.mult, ALU.add)
                    if kt % 2 == 1:
                        ktp = kt // 2
                        for q0 in range(0, SQ, 512):
                            nc.tensor.matmul(
                                ctx_ps[0:VW, q0:q0 + 512],
                                v_sb[:, ktp, :, h * VW:(h + 1) * VW],
                                pt[:, :, q0:q0 + 512],
                                start=(ktp == 0), stop=(ktp == NKTP - 1),
                                perf_mode=PM.DoubleRow)
                # normalize + evict context (x64 / denominator)
                rc = recips.tile([1, SQ], F32, tag="rc", name=f"rc_h{h}")
                nc.vector.reciprocal(rc[:], ctx_ps[64:65, :])
                rb = recips.tile([D_K, SQ], F32, tag="rb", name=f"rb_h{h}")
                nc.gpsimd.partition_broadcast(rb[:], rc[:], channels=D_K)
                if h == 0:
                    ctx_sb = acts.tile([D_K, 4, 2, SQ], FP8E4, tag="ctx")
                nc.vector.scalar_tensor_tensor(
                    ctx_sb[:, h // 2, h % 2, :], ctx_ps[0:D_K, :],
                    CTX_SCALE, rb[:], ALU.mult, ALU.mult)

            # ---- output projection + residual + layernorm -----------------
            for qc in range(NQC):
                zps = ps_sc.tile([P, D_MODEL], F32, tag="sc", name=f"z_{qc}")
                for pair in range(4):
                    nc.tensor.matmul(
                        zps[:], ctx_sb[:, pair, :, qc * P:(qc + 1) * P],
                        wo_sb[:, pair, :, :],
                        start=(pair == 0), stop=(pair == 3),
                        perf_mode=PM.DoubleRow)
                z = lnp.tile([P, D_MODEL], F32, tag="z", name=f"zz_{qc}")
                nc.vector.scalar_tensor_tensor(
                    z[:], zps[:], 1.0 / CTX_SCALE, qres_sb[:, qc, :],
                    ALU.mult, ALU.add)
                stats = small.tile([P, 6], F32, tag="stats")
                nc.vector.bn_stats(stats[:], z[:])
                mv = small.tile([P, 2], F32, tag="mv")
                nc.vector.bn_aggr(mv[:], stats[:])
                istd = small.tile([P, 1], F32, tag="istd")
                nc.scalar.activation(istd[:], mv[:, 1:2], AF.Sqrt,
                                     bias=eps_sb[:], scale=1.0)
                nc.vector.reciprocal(istd[:], istd[:])
                zo = lnp.tile([P, D_MODEL], F32, tag="zo", name=f"zo_{qc}")
                nc.gpsimd.tensor_scalar(zo[:], z[:], mv[:, 0:1], istd[:],
                                        ALU.subtract, ALU.mult)
                if ln_affine:
                    nc.gpsimd.tensor_tensor(zo[:], zo[:], gam_sb[:], ALU.mult)
                    nc.gpsimd.tensor_tensor(zo[:], zo[:], bet_sb[:], ALU.add)
                nc.sync.dma_start(out[qc * P:(qc + 1) * P, :], zo[:])

    nc.compile()
    return nc


def _get_nc(qkv_bias: bool, ln_affine: bool):
    key = (qkv_bias, ln_affine)
    if key not in _NC_CACHE:
        _NC_CACHE[key] = _build(*key)
    return _NC_CACHE[key]


def _dr_input(x):
    """[seq, 512] f32 -> [128, s2, cp2, seq] e4m3 (c = 256s+128cp+p)."""
    return np.ascontiguousarray(
        x.T.reshape(2, 2, P, -1).transpose(2, 0, 1, 3)).astype(E4)


def _dr_weight(W):
    """W [512, 512] -> W.T as [128, s2, cp2, 512] e4m3 (c = 256s+128cp+p)."""
    return np.ascontiguousarray(
        W.T.reshape(2, 2, P, D_MODEL).transpose(2, 0, 1, 3)).astype(E4)


def prepare(query, key, value, key_mask, Wq, bq, Wk, bk, Wv, bv, Wo, bo,
            ln_gamma, ln_beta):
    query = np.asarray(query, dtype=np.float32)
    key = np.asarray(key, dtype=np.float32)
    value = np.asarray(value, dtype=np.float32)
    key_mask = np.asarray(key_mask)
    Wq = np.asarray(Wq, dtype=np.float32)
    Wk = np.asarray(Wk, dtype=np.float32)
    Wv = np.asarray(Wv, dtype=np.float32)
    Wo = np.asarray(Wo, dtype=np.float32)
    bq = np.asarray(bq, dtype=np.float32)
    bk = np.asarray(bk, dtype=np.float32)
    bv = np.asarray(bv, dtype=np.float32)
    bo = np.asarray(bo, dtype=np.float32)
    ln_gamma = np.asarray(ln_gamma, dtype=np.float32)
    ln_beta = np.asarray(ln_beta, dtype=np.float32)

    B, sq_full, dm = query.shape
    assert (B, sq_full, dm) == (4, 2048, 512), query.shape

    qkv_bias = bool(bq.any() or bk.any() or bv.any())
    ln_affine = bool((ln_gamma != 1.0).any() or ln_beta.any())
    nc = _get_nc(qkv_bias, ln_affine)

    wq_dr = _dr_weight(Wq)
    wk_dr = _dr_weight(Wk)
    wv_dr = _dr_weight(Wv)
    wo_dr = np.ascontiguousarray(
        Wo.T.reshape(4, 2, D_K, D_MODEL).transpose(2, 0, 1, 3)).astype(E4)

    qres_full = query + bo[None, None, :]

    per_batch = {}
    for b in range(B):
        m = key_mask[b].reshape(16, P).T.astype(np.float32)  # [p, kt]
        per_batch[b] = {
            "kT": _dr_input(key[b]),
            "vT": _dr_input(value[b]),
            "at": np.ascontiguousarray(m * SCH_A),
            "bt": np.ascontiguousarray(m * SCH_B),
            "mt": np.ascontiguousarray((1.0 - m) * ACT_MASK_BIAS),
        }

    in_maps = []
    for core in range(N_CORES):
        b, half = divmod(core, 2)
        rows = slice(half * SQ, (half + 1) * SQ)
        m = {
            "qT": _dr_input(query[b, rows]),
            "qres": np.ascontiguousarray(qres_full[b, rows]),
            "wq": wq_dr, "wk": wk_dr, "wv": wv_dr, "wo": wo_dr,
            **per_batch[b],
        }
        if qkv_bias:
            m["bqp"] = np.ascontiguousarray(bq.reshape(4, P).T).astype(np.float32)
            m["bkp"] = np.ascontiguousarray(bk.reshape(4, P).T).astype(np.float32)
            m["bvb"] = np.ascontiguousarray(
                np.broadcast_to(bv, (P, D_MODEL))).astype(np.float32)
        if ln_affine:
            m["gam"] = np.ascontiguousarray(
                np.broadcast_to(ln_gamma, (P, D_MODEL))).astype(np.float32)
            m["bet"] = np.ascontiguousarray(
                np.broadcast_to(ln_beta, (P, D_MODEL))).astype(np.float32)
        in_maps.append(m)
    return nc, in_maps


def kernel(**inputs):
    nc, in_maps = prepare(**inputs)
    B, sq_full, dm = 4, 2048, 512

    res = bass_utils.run_bass_kernel_spmd(nc, in_maps,
                                          core_ids=list(range(N_CORES)))
    out = np.empty((B, sq_full, dm), dtype=np.float32)
    for core in range(N_CORES):
        b, half = divmod(core, 2)
        out[b, half * SQ:(half + 1) * SQ] = res.results[core]["out"]
    return out


# revision 24
# speedup vs baseline: 1.3076x; 1.3076x over previous
"""Trainium2 Bass kernel for a cross-attention transformer layer.

Reference computation (per batch b):
    Q = query @ Wq.T ; K = key @ Wk.T ; V = value @ Wv.T
    scores = QK^T/sqrt(d_k) per head, masked, softmax
    out = LayerNorm(softmax(scores) V @ Wo.T + query)

Sharding: 8 cores = 4 batches x 2 query-halves; no collectives.

All matmuls run in fp8 (e4m3 operands) DoubleRow perf mode: each matmul
contracts 2x128 (two "planes" in the free dim of both operands) at 0.5
cycles per output column. Layouts (per core, device side):
  qT/kT/vT inputs  [128, s2, cp2, seq]   c_in = 256*s + 128*cp + p
  wq/wk/wv (DR)    [128, s2, cp2, 512]   plain chunked W.T
  QT/KT            [128, g4, seq] bf16; partition 64*(h%2)+dk, g = h//2.
                   Scores are plain bf16 matmuls (K=64, base 0/64; DR is
                   impossible for a 64-deep contraction since matmul
                   operand base partitions are restricted to {0,32,64})
  V (+ones col)    [128, ktp8, vpl2, h*65+e]  ctx DR contracts 2 k-tiles
                   per step; the ones column makes psum row 64 the softmax
                   denominator for free
  PT = exp(s/8)    e5m2 [128, 2, 1024] per k-tile pair; written either by
                   ACT (table exp, fp8 out) or DVE (Schraudolph: the int8
                   of round(s*0.72135 + 59.75) IS the e5m2 bit pattern;
                   mask rides in the per-partition scale/offset tables)
  ctx              [64, pair4, h2, q] e4m3, scaled x64/denominator at
                   eviction (Pool broadcasts 1/den across partitions)
  O-proj           DR over pairs (K=64 x 2 head planes), /64 folded into
                   the residual add; LayerNorm: DVE stats + Pool apply.
"""

import sys

if "/opt/trn_rl_repo" not in sys.path:
    sys.path.insert(0, "/opt/trn_rl_repo")

import numpy as np
import ml_dtypes

import concourse.bacc as bacc
import concourse.mybir as mybir
import concourse.tile as tile
from concourse import bass_utils

F32 = mybir.dt.float32
BF16 = mybir.dt.bfloat16
FP8E4 = mybir.dt.float8e4
FP8E5 = mybir.dt.float8e5
I8 = mybir.dt.int8
AF = mybir.ActivationFunctionType
ALU = mybir.AluOpType
PM = mybir.MatmulPerfMode

E4 = ml_dtypes.float8_e4m3
E5 = ml_dtypes.float8_e5m2

D_MODEL = 512
N_HEADS = 8
D_K = 64
SQ = 1024          # query rows per core
SK = 2048          # key rows per core
N_CORES = 8
P = 128

# Schraudolph constants for e5m2 bits: round(x*(4/ln2)*0.125 + 60 - C)
SCH_A = 0.125 * 4.0 / np.log(2.0)
SCH_B = 59.75
ACT_MASK_BIAS = -30.0          # exp(-30) underflows e5m2 -> exact 0
CTX_SCALE = 64.0

# exp engine assignment per kt (per head): True -> ACT, False -> DVE.
# Interleaved within kt-pairs so both engines work concurrently; ACT gets
# one extra pair since its per-tile exp is ~15% faster than DVE's.
EXP_ON_ACT = [True, False] * 7 + [True, True]

_NC_CACHE: dict = {}


def _build(qkv_bias: bool, ln_affine: bool):
    nc = bacc.Bacc("TRN2", target_bir_lowering=False, debug=False,
                   enable_asserts=False, num_devices=N_CORES)

    d = lambda name, shape, dt: nc.dram_tensor(name, shape, dt, kind="ExternalInput").ap()
    qT = d("qT", [P, 2, 2, SQ], FP8E4)
    kT = d("kT", [P, 2, 2, SK], FP8E4)
    vT = d("vT", [P, 2, 2, SK], FP8E4)
    wq = d("wq", [P, 2, 2, D_MODEL], FP8E4)
    wk = d("wk", [P, 2, 2, D_MODEL], FP8E4)
    wv = d("wv", [P, 2, 2, D_MODEL], FP8E4)
    wo = d("wo", [D_K, 4, 2, D_MODEL], FP8E4)
    at = d("at", [P, 16], F32)       # schraudolph scale per (key, kt)
    bt = d("bt", [P, 16], F32)       # schraudolph offset per (key, kt)
    mt = d("mt", [P, 16], F32)       # ACT exp bias (mask)
    qres = d("qres", [SQ, D_MODEL], F32)
    if qkv_bias:
        bqp = d("bqp", [P, 4], F32)      # bias chunked to QT partition order
        bkp = d("bkp", [P, 4], F32)
        bvb = d("bvb", [P, D_MODEL], F32)
    if ln_affine:
        gam = d("gam", [P, D_MODEL], F32)
        bet = d("bet", [P, D_MODEL], F32)
    out = nc.dram_tensor("out", [SQ, D_MODEL], F32, kind="ExternalOutput").ap()

    NQC = SQ // P      # 8 q chunks for O-proj/LN
    KT_TILES = SK // P  # 16
    NKTP = KT_TILES // 2

    with tile.TileContext(nc) as tc:
        with (
            tc.tile_pool(name="consts", bufs=1) as consts,
            tc.tile_pool(name="inbuf", bufs=1) as inbuf,
            tc.tile_pool(name="acts", bufs=1) as acts,
            tc.tile_pool(name="ptp", bufs=4) as ptp,
            tc.tile_pool(name="nrm", bufs=2) as nrm,
            tc.tile_pool(name="lnp", bufs=2) as lnp,
            tc.tile_pool(name="small", bufs=4) as small,
            # PSUM: "sc" 3x[128,1024]f32 = 6 banks, "ctx" 1x = 2 banks
            tc.tile_pool(name="ps_sc", bufs=3, space="PSUM") as ps_sc,
            tc.tile_pool(name="ps_ctx", bufs=1, space="PSUM") as ps_ctx,
        ):
            # ---- loads ----------------------------------------------------
            wq_sb = consts.tile([P, 2, 2, D_MODEL], FP8E4, tag="wq")
            nc.sync.dma_start(wq_sb[:], wq)
            qT_sb = inbuf.tile([P, 2, 2, SQ], FP8E4, tag="qT")
            nc.sync.dma_start(qT_sb[:], qT)
            at_sb = consts.tile([P, 16], F32, tag="at")
            nc.sync.dma_start(at_sb[:], at)
            bt_sb = consts.tile([P, 16], F32, tag="bt")
            nc.sync.dma_start(bt_sb[:], bt)
            mt_sb = consts.tile([P, 16], F32, tag="mt")
            nc.sync.dma_start(mt_sb[:], mt)
            wk_sb = consts.tile([P, 2, 2, D_MODEL], FP8E4, tag="wk")
            nc.sync.dma_start(wk_sb[:], wk)
            kT_sb = inbuf.tile([P, 2, 2, SK], FP8E4, tag="kT")
            nc.sync.dma_start(kT_sb[:], kT)
            wv_sb = consts.tile([P, 2, 2, D_MODEL], FP8E4, tag="wv")
            nc.sync.dma_start(wv_sb[:], wv)
            vT_sb = inbuf.tile([P, 2, 2, SK], FP8E4, tag="vT")
            nc.sync.dma_start(vT_sb[:], vT)
            wo_sb = consts.tile([D_K, 4, 2, D_MODEL], FP8E4, tag="wo")
            nc.sync.dma_start(wo_sb[:], wo)
            qres_sb = inbuf.tile([P, NQC, D_MODEL], F32, tag="qres")
            nc.sync.dma_start(qres_sb[:],
                              qres.rearrange("(qc p) o -> p qc o", p=P))
            if qkv_bias:
                bqp_sb = consts.tile([P, 4], F32, tag="bqp")
                nc.sync.dma_start(bqp_sb[:], bqp)
                bkp_sb = consts.tile([P, 4], F32, tag="bkp")
                nc.sync.dma_start(bkp_sb[:], bkp)
                bvb_sb = consts.tile([P, D_MODEL], F32, tag="bvb")
                nc.sync.dma_start(bvb_sb[:], bvb)
            if ln_affine:
                gam_sb = consts.tile([P, D_MODEL], F32, tag="gam")
                nc.sync.dma_start(gam_sb[:], gam)
                bet_sb = consts.tile([P, D_MODEL], F32, tag="bet")
                nc.sync.dma_start(bet_sb[:], bet)
            eps_sb = consts.tile([P, 1], F32, tag="eps")
            nc.gpsimd.memset(eps_sb[:], 1e-5)

            # V padded to 96 cols/head (DR weights need M % 32 == 0):
            # e<64 = V, e=64 ones (softmax denominator row), e>64 zeros
            VW = 96
            v_sb = acts.tile([P, NKTP, 2, N_HEADS * VW], FP8E4, tag="v")
            v_by_head = v_sb[:].rearrange("p a b (h e) -> p a b h e", e=VW)
            nc.gpsimd.memset(v_by_head[:, :, :, :, 64:65], 1.0)
            nc.gpsimd.memset(v_by_head[:, :, :, :, 65:VW], 0.0)

            evict_tick = [0]

            def evict(dst, src, bias_ap=None):
                """PSUM f32 -> SBUF convert, alternating ACT/DVE."""
                use_act = evict_tick[0] % 2 == 0
                evict_tick[0] += 1
                if bias_ap is None:
                    if use_act:
                        nc.scalar.copy(dst, src)
                    else:
                        nc.vector.tensor_copy(dst, src)
                else:
                    if use_act:
                        nc.scalar.activation(dst, src, AF.Identity, bias=bias_ap)
                    else:
                        nc.vector.tensor_scalar_add(dst, src, bias_ap)

            # ---- projections (fp8 DR, M=128) ------------------------------
            QT_s = acts.tile([P, 4, SQ], BF16, tag="QT")
            for g in range(4):
                for q0 in range(0, SQ, 512):
                    psq = ps_sc.tile([P, 512], F32, tag="sc", name="psq")
                    for s in range(2):
                        nc.tensor.matmul(psq[:], wq_sb[:, s, :, g * P:(g + 1) * P],
                                         qT_sb[:, s, :, q0:q0 + 512],
                                         start=(s == 0), stop=(s == 1),
                                         perf_mode=PM.DoubleRow)
                    evict(QT_s[:, g, q0:q0 + 512], psq[:],
                          bqp_sb[:, g:g + 1] if qkv_bias else None)

            KT_s = acts.tile([P, 4, SK], BF16, tag="KT")
            for g in range(4):
                for k0 in range(0, SK, 512):
                    psk = ps_sc.tile([P, 512], F32, tag="sc", name="psk")
                    for s in range(2):
                        nc.tensor.matmul(psk[:], wk_sb[:, s, :, g * P:(g + 1) * P],
                                         kT_sb[:, s, :, k0:k0 + 512],
                                         start=(s == 0), stop=(s == 1),
                                         perf_mode=PM.DoubleRow)
                    evict(KT_s[:, g, k0:k0 + 512], psk[:],
                          bkp_sb[:, g:g + 1] if qkv_bias else None)

            for kt in range(KT_TILES):
                psv = ps_sc.tile([P, 512], F32, tag="sc", name="psv")
                for s in range(2):
                    nc.tensor.matmul(psv[:], vT_sb[:, s, :, kt * P:(kt + 1) * P],
                                     wv_sb[:, s, :, :],
                                     start=(s == 0), stop=(s == 1),
                                     perf_mode=PM.DoubleRow)
                dst = v_sb[:, kt // 2, kt % 2, :].rearrange(
                    "p (h e) -> p h e", e=VW)[:, :, 0:64]
                src = psv[:].rearrange("p (h e) -> p h e", e=64)
                if qkv_bias:
                    nc.vector.scalar_tensor_tensor(
                        dst, src, 1.0,
                        bvb_sb[:].rearrange("p (h e) -> p h e", e=64),
                        ALU.mult, ALU.add)
                else:
                    evict(dst, src)

            # ---- attention ------------------------------------------------
            recips = nrm  # [1,1024] f32 + [64,1024] f32 tiles
            ctx_sb = acts.tile([D_K, 4, 2, SQ], FP8E4, tag="ctx")
            pending = None  # 1-pair software pipeline on the PE stream

            def emit_ctx(h, ktp, ctx_ps, pt):
                for q0 in range(0, SQ, 512):
                    nc.tensor.matmul(
                        ctx_ps[0:VW, q0:q0 + 512],
                        v_sb[:, ktp, :, h * VW:(h + 1) * VW],
                        pt[:, :, q0:q0 + 512],
                        start=(ktp == 0), stop=(ktp == NKTP - 1),
                        perf_mode=PM.DoubleRow)

            def finish_head(h, ctx_ps):
                # normalize + evict context (x64 / denominator in psum row 64)
                rc = recips.tile([1, SQ], F32, tag="rc", name=f"rc_h{h}")
                nc.vector.reciprocal(rc[:], ctx_ps[64:65, :])
                rb = recips.tile([D_K, SQ], F32, tag="rb", name=f"rb_h{h}")
                nc.gpsimd.partition_broadcast(rb[:], rc[:], channels=D_K)
                nc.vector.scalar_tensor_tensor(
                    ctx_sb[:, h // 2, h % 2, :], ctx_ps[0:D_K, :],
                    CTX_SCALE, rb[:], ALU.mult, ALU.mult)

            ctx_ps = None
            for h in range(N_HEADS):
                g, hb = h // 2, h % 2
                b0 = D_K * hb
                pt = None
                for kt in range(KT_TILES):
                    if kt % 2 == 0:
                        pt = ptp.tile([P, 2, SQ], FP8E5, tag="pt",
                                      name=f"pt_h{h}_{kt // 2}")
                    sc = ps_sc.tile([P, SQ], F32, tag="sc", name=f"sc_h{h}_{kt}")
                    for q0 in range(0, SQ, 512):
                        nc.tensor.matmul(
                            sc[:, q0:q0 + 512],
                            KT_s[b0:b0 + D_K, g, kt * P:(kt + 1) * P],
                            QT_s[b0:b0 + D_K, g, q0:q0 + 512],
                            start=True, stop=True)
                    dst = pt[:, kt % 2, :]
                    if EXP_ON_ACT[kt]:
                        nc.scalar.activation(dst, sc[:], AF.Exp,
                                             bias=mt_sb[:, kt:kt + 1],
                                             scale=0.125)
                    else:
                        nc.vector.tensor_scalar(
                            dst.bitcast(I8), sc[:],
                            at_sb[:, kt:kt + 1], bt_sb[:, kt:kt + 1],
                            ALU.mult, ALU.add)
                    if kt % 2 == 1:
                        if pending is not None:
                            ph, pktp, pctx, ppt = pending
                            emit_ctx(ph, pktp, pctx, ppt)
                            if pktp == NKTP - 1:
                                finish_head(ph, pctx)
                        if kt == 1:
                            # allocate after the previous head's flush +
                            # eviction so the bufs=1 slot-reuse dependency
                            # covers them
                            ctx_ps = ps_ctx.tile([P, SQ], F32, tag="ctx",
                                                 name=f"ctx_h{h}")
                        pending = (h, kt // 2, ctx_ps, pt)
            ph, pktp, pctx, ppt = pending
            emit_ctx(ph, pktp, pctx, ppt)
            finish_head(ph, pctx)

            # ---- output projection + residual + layernorm -----------------
            for qc in range(NQC):
                zps = ps_sc.tile([P, D_MODEL], F32, tag="sc", name=f"z_{qc}")
                for pair in range(4):
                    nc.tensor.matmul(
                        zps[:], ctx_sb[:, pair, :, qc * P:(qc + 1) * P],
                        wo_sb[:, pair, :, :],
                        start=(pair == 0), stop=(pair == 3),
                        perf_mode=PM.DoubleRow)
                z = lnp.tile([P, D_MODEL], F32, tag="z", name=f"zz_{qc}")
                nc.vector.scalar_tensor_tensor(
                    z[:], zps[:], 1.0 / CTX_SCALE, qres_sb[:, qc, :],
                    ALU.mult, ALU.add)
                stats = small.tile([P, 6], F32, tag="stats")
                nc.vector.bn_stats(stats[:], z[:])
                mv = small.tile([P, 2], F32, tag="mv")
                nc.vector.bn_aggr(mv[:], stats[:])
                istd = small.tile([P, 1], F32, tag="istd")
                nc.scalar.activation(istd[:], mv[:, 1:2], AF.Sqrt,
                                     bias=eps_sb[:], scale=1.0)
                nc.vector.reciprocal(istd[:], istd[:])
                zo = lnp.tile([P, D_MODEL], F32, tag="zo", name=f"zo_{qc}")
                nc.gpsimd.tensor_scalar(zo[:], z[:], mv[:, 0:1], istd[:],
                                        ALU.subtract, ALU.mult)
                if ln_affine:
                    nc.gpsimd.tensor_tensor(zo[:], zo[:], gam_sb[:], ALU.mult)
                    nc.gpsimd.tensor_tensor(zo[:], zo[:], bet_sb[:], ALU.add)
                nc.sync.dma_start(out[qc * P:(qc + 1) * P, :], zo[:])

    nc.compile()
    return nc


def _get_nc(qkv_bias: bool, ln_affine: bool):
    key = (qkv_bias, ln_affine)
    if key not in _NC_CACHE:
        _NC_CACHE[key] = _build(*key)
    return _NC_CACHE[key]


def _dr_input(x):
    """[seq, 512] f32 -> [128, s2, cp2, seq] e4m3 (c = 256s+128cp+p)."""
    return np.ascontiguousarray(
        x.T.reshape(2, 2, P, -1).transpose(2, 0, 1, 3)).astype(E4)


def _dr_weight(W):
    """W [512, 512] -> W.T as [128, s2, cp2, 512] e4m3 (c = 256s+128cp+p)."""
    return np.ascontiguousarray(
        W.T.reshape(2, 2, P, D_MODEL).transpose(2, 0, 1, 3)).astype(E4)


def prepare(query, key, value, key_mask, Wq, bq, Wk, bk, Wv, bv, Wo, bo,
            ln_gamma, ln_beta):
    query = np.asarray(query, dtype=np.float32)
    key = np.asarray(key, dtype=np.float32)
    value = np.asarray(value, dtype=np.float32)
    key_mask = np.asarray(key_mask)
    Wq = np.asarray(Wq, dtype=np.float32)
    Wk = np.asarray(Wk, dtype=np.float32)
    Wv = np.asarray(Wv, dtype=np.float32)
    Wo = np.asarray(Wo, dtype=np.float32)
    bq = np.asarray(bq, dtype=np.float32)
    bk = np.asarray(bk, dtype=np.float32)
    bv = np.asarray(bv, dtype=np.float32)
    bo = np.asarray(bo, dtype=np.float32)
    ln_gamma = np.asarray(ln_gamma, dtype=np.float32)
    ln_beta = np.asarray(ln_beta, dtype=np.float32)

    B, sq_full, dm = query.shape
    assert (B, sq_full, dm) == (4, 2048, 512), query.shape

    qkv_bias = bool(bq.any() or bk.any() or bv.any())
    ln_affine = bool((ln_gamma != 1.0).any() or ln_beta.any())
    nc = _get_nc(qkv_bias, ln_affine)

    wq_dr = _dr_weight(Wq)
    wk_dr = _dr_weight(Wk)
    wv_dr = _dr_weight(Wv)
    wo_dr = np.ascontiguousarray(
        Wo.T.reshape(4, 2, D_K, D_MODEL).transpose(2, 0, 1, 3)).astype(E4)

    qres_full = query + bo[None, None, :]

    per_batch = {}
    for b in range(B):
        m = key_mask[b].reshape(16, P).T.astype(np.float32)  # [p, kt]
        per_batch[b] = {
            "kT": _dr_input(key[b]),
            "vT": _dr_input(value[b]),
            "at": np.ascontiguousarray(m * SCH_A),
            "bt": np.ascontiguousarray(m * SCH_B),
            "mt": np.ascontiguousarray((1.0 - m) * ACT_MASK_BIAS),
        }

    in_maps = []
    for core in range(N_CORES):
        b, half = divmod(core, 2)
        rows = slice(half * SQ, (half + 1) * SQ)
        m = {
            "qT": _dr_input(query[b, rows]),
            "qres": np.ascontiguousarray(qres_full[b, rows]),
            "wq": wq_dr, "wk": wk_dr, "wv": wv_dr, "wo": wo_dr,
            **per_batch[b],
        }
        if qkv_bias:
            m["bqp"] = np.ascontiguousarray(bq.reshape(4, P).T).astype(np.float32)
            m["bkp"] = np.ascontiguousarray(bk.reshape(4, P).T).astype(np.float32)
            m["bvb"] = np.ascontiguousarray(
                np.broadcast_to(bv, (P, D_MODEL))).astype(np.float32)
        if ln_affine:
            m["gam"] = np.ascontiguousarray(
                np.broadcast_to(ln_gamma, (P, D_MODEL))).astype(np.float32)
            m["bet"] = np.ascontiguousarray(
                np.broadcast_to(ln_beta, (P, D_MODEL))).astype(np.float32)
        in_maps.append(m)
    return nc, in_maps


def kernel(**inputs):
    nc, in_maps = prepare(**inputs)
    B, sq_full, dm = 4, 2048, 512

    res = bass_utils.run_bass_kernel_spmd(nc, in_maps,
                                          core_ids=list(range(N_CORES)))
    out = np.empty((B, sq_full, dm), dtype=np.float32)
    for core in range(N_CORES):
        b, half = divmod(core, 2)
        out[b, half * SQ:(half + 1) * SQ] = res.results[core]["out"]
    return out
